# revision 12
# baseline (speedup 1.0000x reference)
"""Trainium2 Bass kernel for nn_MHA_75110388072824.

Multi-head attention, B=2, T=2048, D=2048, NH=16 heads (hd=128), fp32,
causal mask, y = softmax(mask((x Wq^T)(x Wk^T)^T / sqrt(hd))) (x Wv^T) Wo^T.

Sharding over 8 NeuronCores: core = b*4 + hg, b in {0,1} batch,
hg in {0..3} a group of 4 heads (tensor-parallel columns of Wq/Wk/Wv,
rows of Wo).  Each core computes a partial output [T, D] = Z_hg @ Wo_hg^T;
the host sums the 4 head-group partials per batch.

Per-core pipeline (all matmuls in float32r = full-speed TF32-like):
  Phase A: stream x^T by 512-column chunks, weights resident; compute
           Q^T, K^T (per head, [hd=128 part, T]) and V ([s part, d]) and
           spill to DRAM scratch.
  Phase B: per head: reload Q^T/K^T (one DMA) + V slice; per 512-wide
           t-chunk: S = Q^T-block^T @ K^T (causal chunks only), additive
           mask on diagonal chunks, exp on ACT with fused row-sum
           (accum_out), normalize rows by 1/l on ACT, PE-transpose P
           blocks into [s, t] strips, PV matmuls accumulate Z^T.
  Phase C: out = Z^T^T @ Wo_hg^T via 4 head k-tiles, evicted to DRAM.
"""
import ml_dtypes
import numpy as np

import concourse.bass as bass
import concourse.mybir as mybir
import concourse.tile as tile
from concourse import bacc
from concourse.bass_utils import run_bass_kernel_spmd

P = 128
T = 2048
D = 2048
NH = 16
HPC = 4            # heads per core
HD = 128
NT = T // P        # 16 t-blocks
NC4 = T // 512     # 4 512-chunks
KT = D // P        # 16 k-tiles over D
SCALE = 1.0 / float(np.sqrt(HD))
NEG = np.float32(-3.0e38)
F32 = mybir.dt.float32
F32R = mybir.dt.float32r
BF16 = mybir.dt.bfloat16
EXP = mybir.ActivationFunctionType.Exp
ADD = mybir.AluOpType.add
AXX = mybir.AxisListType.X

# Tunable buffer counts (swept via TimelineSim)
CFG = dict(xs=2, aev=4, aps=8, qk=2, vh=2, strip=5, sps=5, lps=1, zps=2,
           ztev=3, cev=8, cps=4, lr=6,
           a_skip_out=False, a_skip_in=False)


def _phase_a(tc, nc, xT, wqT, wkT, wvT, qk_scr, v_scr, qk0, vh0,
             mask_t, masks):
    if True:
        # ---------------- Phase A: QKV projections -> DRAM scratch (h1-3)
        # and SBUF (head 0).  Psum groups split by weight k-chunk so PE work
        # starts as soon as the first weight chunk lands.
        with (
            tc.tile_pool(name="wqkv", bufs=1) as wpool,
            tc.tile_pool(name="xs", bufs=CFG["xs"]) as xpool,
            tc.tile_pool(name="aev", bufs=CFG["aev"]) as aev,
            tc.tile_pool(name="aps", bufs=CFG["aps"], space="PSUM") as aps,
        ):
            wq_s = wpool.tile([P, KT, 512], F32R, tag="wq")
            wk_s = wpool.tile([P, KT, 512], F32R, tag="wk")
            wv_s = wpool.tile([P, KT, 512], F32R, tag="wv")
            xTr = xT.rearrange("(ko p) t -> p ko t", p=P)
            xs0 = xpool.tile([P, KT, 512], F32R, tag="xs", name="xs0")
            wqTr = wqT.rearrange("(ko p) d -> p ko d", p=P)
            wkTr = wkT.rearrange("(ko p) d -> p ko d", p=P)
            wvTr = wvT.rearrange("(ko p) d -> p ko d", p=P)
            # interleave so Q-pass kc has xs0[kc]+wq[kc] asap; K then V follow
            for kc in range(4):
                ksl = slice(4 * kc, 4 * (kc + 1))
                nc.scalar.dma_start(xs0[:, ksl], xTr[:, ksl, 0:512])
                nc.sync.dma_start(wq_s[:, ksl], wqTr[:, ksl])
            for kc in range(4):
                ksl = slice(4 * kc, 4 * (kc + 1))
                nc.sync.dma_start(wk_s[:, ksl], wkTr[:, ksl])
            # mask load rides the sync queue after wk so it cannot contend
            # with startup-critical DMAs yet lands before phase B needs it
            nc.sync.dma_start(mask_t[:], masks)
            for kc in range(4):
                ksl = slice(4 * kc, 4 * (kc + 1))
                nc.scalar.dma_start(wv_s[:, ksl], wvTr[:, ksl])

            for tci in range(NC4):
                if tci == 0:
                    xs = xs0
                else:
                    xs = xpool.tile([P, KT, 512], F32R, tag="xs")
                    for kc in range(4):
                        ksl = slice(4 * kc, 4 * (kc + 1))
                        nc.scalar.dma_start(
                            xs[:, ksl],
                            xTr[:, ksl, 512 * tci:512 * (tci + 1)])
                # Q then K then V; within each, 4 psum groups accumulate over
                # k-chunks in arrival order.
                for w_s, off in ((wq_s, 0), (wk_s, T)):
                    pss = [aps.tile([P, 512], F32, tag="ps", name=f"ps{i}")
                           for i in range(HPC)]
                    for kc in range(4):
                        for h in range(HPC):
                            for k in range(4 * kc, 4 * kc + 4):
                                nc.tensor.matmul(
                                    pss[h][:], w_s[:, k, h * P:(h + 1) * P],
                                    xs[:, k, :],
                                    start=(k == 0), stop=(k == KT - 1))
                    for h in range(HPC):
                        if h == 0:
                            nc.vector.tensor_copy(
                                qk0[:, off + 512 * tci:off + 512 * (tci + 1)],
                                pss[h][:])
                        else:
                            ev = aev.tile([P, 512], F32R, tag="ev")
                            nc.vector.tensor_copy(ev[:], pss[h][:])
                            nc.sync.dma_start(
                                qk_scr[h, :, off + 512 * tci:off + 512 * (tci + 1)],
                                ev[:])
                pss = [aps.tile([P, 512], F32, tag="ps", name=f"psv{i}")
                       for i in range(HPC)]
                for kc in range(4):
                    for sb in range(4):
                        for k in range(4 * kc, 4 * kc + 4):
                            nc.tensor.matmul(
                                pss[sb][:], xs[:, k, sb * P:(sb + 1) * P],
                                wv_s[:, k, :],
                                start=(k == 0), stop=(k == KT - 1))
                for sb in range(4):
                    st = 4 * tci + sb
                    ev = aev.tile([P, 512], BF16, tag="vev", name="vev")
                    nc.vector.tensor_copy(ev[:], pss[sb][:])
                    nc.vector.tensor_copy(vh0[:, st, :], pss[sb][:, 0:P])
                    nc.sync.dma_start(v_scr[st], ev[:])


def _phase_bc(tc, nc, mask_t, ones_sq, qk_scr, v_scr, woT, out, qk0, vh0):
    if True:
        # ---------------- Phase B+C: attention per head, then output proj.
        # ZT stays in SBUF; Wo preloaded early so C pipelines into B's tail.
        with (
            tc.tile_pool(name="zt", bufs=1) as ztpool,
            tc.tile_pool(name="wo", bufs=1) as wopool,
            tc.tile_pool(name="cev", bufs=CFG["cev"]) as cev,
        ):
            wo_s = wopool.tile([P, HPC, T], F32R)
            woTr = woT.rearrange("(ko p) d -> p ko d", p=P)
            zt_tiles = [ztpool.tile([P, T], F32R, tag=f"zt{h}", name=f"zt{h}")
                        for h in range(HPC)]

            with (
                tc.tile_pool(name="qk", bufs=CFG["qk"]) as qkpool,
                tc.tile_pool(name="vh", bufs=CFG["vh"]) as vhpool,
                tc.tile_pool(name="strip", bufs=CFG["strip"]) as spool,
                tc.tile_pool(name="lr", bufs=CFG["lr"]) as lrpool,
                tc.tile_pool(name="sps", bufs=CFG["sps"], space="PSUM") as sps,
                tc.tile_pool(name="lps", bufs=CFG["lps"], space="PSUM") as lps,
                tc.tile_pool(name="zps", bufs=CFG["zps"], space="PSUM") as zps,
            ):
                v_scr_r = v_scr.rearrange("a p d -> p a d")
                for h in range(HPC):
                    if h == 0:
                        qk, vh = qk0, vh0
                    else:
                        qk = qkpool.tile([P, 2 * T], F32R, tag="qk")
                        # interleave Q/K halves in consumption order: tc2=0
                        # needs Q[0:512] + K[0:512] first
                        nc.sync.dma_start(qk[:, 0:1024], qk_scr[h, :, 0:1024])
                        nc.sync.dma_start(qk[:, T:T + 1024],
                                          qk_scr[h, :, T:T + 1024])
                        nc.sync.dma_start(qk[:, 1024:T], qk_scr[h, :, 1024:T])
                        nc.sync.dma_start(qk[:, T + 1024:],
                                          qk_scr[h, :, T + 1024:])
                        vh = vhpool.tile([P, NT, P], BF16, tag="vh")
                        nc.sync.dma_start(vh[:], v_scr_r[:, :, h * P:(h + 1) * P])
                    nc.sync.dma_start(wo_s[:, h], woTr[:, h])

                    for tc2 in range(4):
                        ns = 4 * tc2 + 4
                        lsum = lps.tile([P, 512], F32, tag="lsum")
                        ztp = zps.tile([P, 512], F32, tag="ztp")
                        for si in range(ns):
                            q = si - 4 * tc2
                            t0 = max(0, 128 * q)   # left edge of valid t range
                            # S is f32r: moving dims <256 run at 1/4 rate, so
                            # clamp S/exp to >=256 wide and let the mask zero
                            # the extra block; lsum/PV are bf16 (no such
                            # penalty) and keep the true t0.
                            t0c = min(t0, 256)
                            sp = sps.tile([P, 512], F32, tag="sp")
                            nc.tensor.matmul(
                                sp[:, t0c:], qk[:, T + si * P:T + (si + 1) * P],
                                qk[:, 512 * tc2 + t0c:512 * (tc2 + 1)],
                                start=True, stop=True)
                            if q >= 0:
                                # only the ambiguous diagonal block (plus the
                                # clamp-added block for q==3) needs masking
                                nc.vector.tensor_tensor(
                                    sp[:, t0c:128 * (q + 1)],
                                    sp[:, t0c:128 * (q + 1)],
                                    mask_t[:, q, t0c:128 * (q + 1)], ADD)
                            strip = spool.tile([P, 512], BF16, tag="strip")
                            nc.scalar.activation(strip[:, t0c:], sp[:, t0c:],
                                                 EXP, bias=0.0, scale=SCALE)
                            nc.tensor.matmul(lsum[:, t0:], ones_sq[:],
                                             strip[:, t0:],
                                             start=(si == 0), stop=(si == ns - 1))
                            nc.tensor.matmul(ztp[:, t0:], vh[:, si, :],
                                             strip[:, t0:],
                                             start=(si == 0), stop=(si == ns - 1))
                        rlb = lrpool.tile([P, 512], F32R, tag="rlb")
                        with nc.allow_low_precision(reason="1/l scale in f32r"):
                            nc.vector.reciprocal(rlb[:], lsum[:])
                        nc.vector.tensor_tensor(
                            zt_tiles[h][:, 512 * tc2:512 * (tc2 + 1)],
                            ztp[:], rlb[:], mybir.AluOpType.mult)

            # ---------------- Phase C: output projection from SBUF ZT
            with (
                tc.tile_pool(name="cps", bufs=CFG["cps"], space="PSUM") as cps,
            ):
                for ti in range(NT):
                    for oc in range(4):
                        ps = cps.tile([P, 512], F32, tag="cps")
                        for h in range(HPC):
                            nc.tensor.matmul(
                                ps[:], zt_tiles[h][:, ti * P:(ti + 1) * P],
                                wo_s[:, h, 512 * oc:512 * (oc + 1)],
                                start=(h == 0), stop=(h == HPC - 1))
                        ev = cev.tile([P, 512], F32, tag="cev")
                        if oc % 2 == 0:
                            nc.vector.tensor_copy(ev[:], ps[:])
                        else:
                            nc.scalar.copy(ev[:], ps[:])
                        nc.sync.dma_start(
                            out[ti * P:(ti + 1) * P, 512 * oc:512 * (oc + 1)],
                            ev[:])


def build(repeat=1, loop_phase=None, phases="ABC"):
    nc = bacc.Bacc("TRN2", target_bir_lowering=False, debug=False)
    xT = nc.dram_tensor("xT", [D, T], F32R, kind="ExternalInput").ap()
    wqT = nc.dram_tensor("wqT", [D, 512], F32R, kind="ExternalInput").ap()
    wkT = nc.dram_tensor("wkT", [D, 512], F32R, kind="ExternalInput").ap()
    wvT = nc.dram_tensor("wvT", [D, 512], F32R, kind="ExternalInput").ap()
    woT = nc.dram_tensor("woT", [512, D], F32R, kind="ExternalInput").ap()
    masks = nc.dram_tensor("masks", [P, 4, 512], BF16,
                           kind="ExternalInput").ap()
    ident = nc.dram_tensor("ident", [P, P], F32, kind="ExternalInput").ap()
    out = nc.dram_tensor("out", [T, D], F32, kind="ExternalOutput").ap()
    qk_scr = nc.dram_tensor("qk_scr", [HPC, P, 2 * T], F32R).ap()
    v_scr = nc.dram_tensor("v_scr", [NT, P, 512], BF16).ap()

    def emit_all():
        with (
            tc.tile_pool(name="h0", bufs=1) as h0pool,
            tc.tile_pool(name="const", bufs=1) as cpool,
        ):
            qk0 = h0pool.tile([P, 2 * T], F32R, name="qk0")
            vh0 = h0pool.tile([P, NT, P], BF16, name="vh0")
            mask_t = cpool.tile([P, 4, 512], BF16)
            ones32 = cpool.tile([P, P], F32)
            nc.vector.memset(ones32[:], 1.0)
            ones_sq = cpool.tile([P, P], BF16)
            nc.vector.tensor_copy(ones_sq[:], ones32[:])
            if "A" in phases:
                _phase_a(tc, nc, xT, wqT, wkT, wvT, qk_scr, v_scr, qk0, vh0,
                         mask_t, masks)
            if "B" in phases:
                _phase_bc(tc, nc, mask_t, ones_sq, qk_scr, v_scr, woT, out,
                          qk0, vh0)

    with tile.TileContext(nc) as tc:
        if repeat == 1 and loop_phase is None:
            emit_all()
        elif loop_phase is None:
            with tc.For_i(0, repeat, 1):
                emit_all()
        else:
            raise ValueError("loop_phase no longer supported")
    nc.compile()
    return nc


def make_inputs(x, Wq, Wk, Wv, Wo):
    """Host-side sharding: returns in_maps for cores 0..7 (core = b*4 + hg)."""
    # transposed mask: strip [s_local, q, t_local]; valid iff sl <= tl - 128*q
    masks = np.full((P, 4, 512), NEG, dtype=np.float32)
    for q in range(4):
        for sl in range(P):
            lo = sl + 128 * q
            if lo < 512:
                masks[sl, q, lo:] = 0.0
    ident = np.eye(P, dtype=np.float32)
    xTs = [np.ascontiguousarray(x[b].T).astype(np.float32) for b in range(2)]
    in_maps = []
    for core in range(8):
        b, hg = core // 4, core % 4
        sl = slice(hg * 512, (hg + 1) * 512)
        in_maps.append({
            "xT": xTs[b],
            "wqT": np.ascontiguousarray(Wq[sl, :].T),
            "wkT": np.ascontiguousarray(Wk[sl, :].T),
            "wvT": np.ascontiguousarray(Wv[sl, :].T),
            "woT": np.ascontiguousarray(Wo[:, sl].T),
            "masks": masks.astype(ml_dtypes.bfloat16),
            "ident": ident,
        })
    return in_maps


_nc_cache = {}


def kernel(x, Wq, Wk, Wv, Wo):
    x = np.asarray(x, dtype=np.float32)
    Wq = np.asarray(Wq, dtype=np.float32)
    Wk = np.asarray(Wk, dtype=np.float32)
    Wv = np.asarray(Wv, dtype=np.float32)
    Wo = np.asarray(Wo, dtype=np.float32)
    if "nc" not in _nc_cache:
        _nc_cache["nc"] = build()
    nc = _nc_cache["nc"]
    in_maps = make_inputs(x, Wq, Wk, Wv, Wo)
    res = run_bass_kernel_spmd(nc, in_maps, core_ids=list(range(8)))
    B = x.shape[0]
    out = np.zeros((B, T, D), dtype=np.float32)
    for core in range(8):
        b = core // 4
        out[b] += res.results[core]["out"]
    return out



# revision 15
# speedup vs baseline: 16.4675x; 16.4675x over previous
"""Trainium2 Bass kernel for nn_MHA_75110388072824.

Multi-head attention, B=2, T=2048, D=2048, NH=16 heads (hd=128), fp32,
causal mask, y = softmax(mask((x Wq^T)(x Wk^T)^T / sqrt(hd))) (x Wv^T) Wo^T.

Sharding over 8 NeuronCores: core = b*4 + hg, b in {0,1} batch,
hg in {0..3} a group of 4 heads (tensor-parallel columns of Wq/Wk/Wv,
rows of Wo).  Each core computes a partial output [T, D] = Z_hg @ Wo_hg^T;
the host sums the 4 head-group partials per batch.

Per-core pipeline (all matmuls in float32r = full-speed TF32-like):
  Phase A: stream x^T by 512-column chunks, weights resident; compute
           Q^T, K^T (per head, [hd=128 part, T]) and V ([s part, d]) and
           spill to DRAM scratch.
  Phase B: per head: reload Q^T/K^T (one DMA) + V slice; per 512-wide
           t-chunk: S = Q^T-block^T @ K^T (causal chunks only), additive
           mask on diagonal chunks, exp on ACT with fused row-sum
           (accum_out), normalize rows by 1/l on ACT, PE-transpose P
           blocks into [s, t] strips, PV matmuls accumulate Z^T.
  Phase C: out = Z^T^T @ Wo_hg^T via 4 head k-tiles, evicted to DRAM.
"""
import ml_dtypes
import numpy as np

import concourse.bass as bass
import concourse.mybir as mybir
import concourse.tile as tile
from concourse import bacc
from concourse.bass_utils import run_bass_kernel_spmd

P = 128
T = 2048
D = 2048
NH = 16
HPC = 4            # heads per core
HD = 128
NT = T // P        # 16 t-blocks
NC4 = T // 512     # 4 512-chunks
KT = D // P        # 16 k-tiles over D
SCALE = 1.0 / float(np.sqrt(HD))
NEG = np.float32(-3.0e38)
F32 = mybir.dt.float32
F32R = mybir.dt.float32r
BF16 = mybir.dt.bfloat16
EXP = mybir.ActivationFunctionType.Exp
ADD = mybir.AluOpType.add
AXX = mybir.AxisListType.X

# Tunable buffer counts (swept via TimelineSim)
CFG = dict(xs=2, aev=4, aps=8, qk=2, vh=2, strip=5, sps=5, lps=1, zps=2,
           ztev=3, cev=8, cps=4, lr=6,
           a_skip_out=False, a_skip_in=False)


def _phase_a(tc, nc, xT, wqT, wkT, wvT, qk_scr, qk0, vh_all,
             mask_t, masks):
    if True:
        # ---------------- Phase A: QKV projections -> DRAM scratch (h1-3)
        # and SBUF (head 0).  Psum groups split by weight k-chunk so PE work
        # starts as soon as the first weight chunk lands.
        with (
            tc.tile_pool(name="wqkv", bufs=1) as wpool,
            tc.tile_pool(name="xs", bufs=CFG["xs"]) as xpool,
            tc.tile_pool(name="aev", bufs=CFG["aev"]) as aev,
            tc.tile_pool(name="aps", bufs=CFG["aps"], space="PSUM") as aps,
        ):
            wq_s = wpool.tile([P, KT, 512], F32R, tag="wq")
            wk_s = wpool.tile([P, KT, 512], F32R, tag="wk")
            wv_s = wpool.tile([P, KT, 512], F32R, tag="wv")
            xTr = xT.rearrange("(ko p) t -> p ko t", p=P)
            xs0 = xpool.tile([P, KT, 512], F32R, tag="xs", name="xs0")
            wqTr = wqT.rearrange("(ko p) d -> p ko d", p=P)
            wkTr = wkT.rearrange("(ko p) d -> p ko d", p=P)
            wvTr = wvT.rearrange("(ko p) d -> p ko d", p=P)
            # interleave so Q-pass kc has xs0[kc]+wq[kc] asap; K then V follow
            for kc in range(4):
                ksl = slice(4 * kc, 4 * (kc + 1))
                nc.scalar.dma_start(xs0[:, ksl], xTr[:, ksl, 0:512])
                nc.sync.dma_start(wq_s[:, ksl], wqTr[:, ksl])
            for kc in range(4):
                ksl = slice(4 * kc, 4 * (kc + 1))
                nc.sync.dma_start(wk_s[:, ksl], wkTr[:, ksl])
            # mask load rides the sync queue after wk so it cannot contend
            # with startup-critical DMAs yet lands before phase B needs it
            nc.sync.dma_start(mask_t[:], masks)
            for kc in range(4):
                ksl = slice(4 * kc, 4 * (kc + 1))
                nc.scalar.dma_start(wv_s[:, ksl], wvTr[:, ksl])

            for tci in range(NC4):
                if tci == 0:
                    xs = xs0
                else:
                    xs = xpool.tile([P, KT, 512], F32R, tag="xs")
                    for kc in range(4):
                        ksl = slice(4 * kc, 4 * (kc + 1))
                        nc.scalar.dma_start(
                            xs[:, ksl],
                            xTr[:, ksl, 512 * tci:512 * (tci + 1)])
                # Q then K then V; within each, 4 psum groups accumulate over
                # k-chunks in arrival order.
                for w_s, off in ((wq_s, 0), (wk_s, T)):
                    pss = [aps.tile([P, 512], F32, tag="ps", name=f"ps{i}")
                           for i in range(HPC)]
                    for kc in range(4):
                        for h in range(HPC):
                            for k in range(4 * kc, 4 * kc + 4):
                                nc.tensor.matmul(
                                    pss[h][:], w_s[:, k, h * P:(h + 1) * P],
                                    xs[:, k, :],
                                    start=(k == 0), stop=(k == KT - 1))
                    for h in range(HPC):
                        if h == 0:
                            nc.vector.tensor_copy(
                                qk0[:, off + 512 * tci:off + 512 * (tci + 1)],
                                pss[h][:])
                        else:
                            ev = aev.tile([P, 512], F32R, tag="ev")
                            nc.vector.tensor_copy(ev[:], pss[h][:])
                            nc.sync.dma_start(
                                qk_scr[h, :, off + 512 * tci:off + 512 * (tci + 1)],
                                ev[:])
                pss = [aps.tile([P, 512], F32, tag="ps", name=f"psv{i}")
                       for i in range(HPC)]
                for kc in range(4):
                    for sb in range(4):
                        for k in range(4 * kc, 4 * kc + 4):
                            nc.tensor.matmul(
                                pss[sb][:], xs[:, k, sb * P:(sb + 1) * P],
                                wv_s[:, k, :],
                                start=(k == 0), stop=(k == KT - 1))
                for sb in range(4):
                    st = 4 * tci + sb
                    for h in range(HPC):
                        nc.vector.tensor_copy(
                            vh_all[h][:, st, :],
                            pss[sb][:, h * P:(h + 1) * P])


def _phase_bc(tc, nc, mask_t, ones_sq, qk_scr, woT, out, qk0, vh_all):
    if True:
        # ---------------- Phase B+C: attention per head, then output proj.
        # ZT stays in SBUF; Wo preloaded early so C pipelines into B's tail.
        with (
            tc.tile_pool(name="zt", bufs=1) as ztpool,
            tc.tile_pool(name="wo", bufs=1) as wopool,
            tc.tile_pool(name="cev", bufs=CFG["cev"]) as cev,
        ):
            wo_s = wopool.tile([P, HPC, T], F32R)
            woTr = woT.rearrange("(ko p) d -> p ko d", p=P)
            zt_tiles = [ztpool.tile([P, T], F32R, tag=f"zt{h}", name=f"zt{h}")
                        for h in range(HPC)]

            with (
                tc.tile_pool(name="qk", bufs=CFG["qk"]) as qkpool,
                tc.tile_pool(name="strip", bufs=CFG["strip"]) as spool,
                tc.tile_pool(name="lr", bufs=CFG["lr"]) as lrpool,
                tc.tile_pool(name="sps", bufs=CFG["sps"], space="PSUM") as sps,
                tc.tile_pool(name="lps", bufs=CFG["lps"], space="PSUM") as lps,
                tc.tile_pool(name="zps", bufs=CFG["zps"], space="PSUM") as zps,
            ):
                for h in range(HPC):
                    vh = vh_all[h]
                    if h == 0:
                        qk = qk0
                    else:
                        qk = qkpool.tile([P, 2 * T], F32R, tag="qk")
                        # interleave Q/K halves in consumption order: tc2=0
                        # needs Q[0:512] + K[0:512] first
                        nc.sync.dma_start(qk[:, 0:1024], qk_scr[h, :, 0:1024])
                        nc.sync.dma_start(qk[:, T:T + 1024],
                                          qk_scr[h, :, T:T + 1024])
                        nc.sync.dma_start(qk[:, 1024:T], qk_scr[h, :, 1024:T])
                        nc.sync.dma_start(qk[:, T + 1024:],
                                          qk_scr[h, :, T + 1024:])
                    nc.sync.dma_start(wo_s[:, h], woTr[:, h])

                    for tc2 in range(4):
                        ns = 4 * tc2 + 4
                        lsum = lps.tile([P, 512], F32, tag="lsum")
                        ztp = zps.tile([P, 512], F32, tag="ztp")
                        for si in range(ns):
                            q = si - 4 * tc2
                            t0 = max(0, 128 * q)   # left edge of valid t range
                            # S is f32r: moving dims <256 run at 1/4 rate, so
                            # clamp S/exp to >=256 wide and let the mask zero
                            # the extra block; lsum/PV are bf16 (no such
                            # penalty) and keep the true t0.
                            t0c = min(t0, 256)
                            sp = sps.tile([P, 512], F32, tag="sp")
                            nc.tensor.matmul(
                                sp[:, t0c:], qk[:, T + si * P:T + (si + 1) * P],
                                qk[:, 512 * tc2 + t0c:512 * (tc2 + 1)],
                                start=True, stop=True)
                            if q >= 0:
                                # only the ambiguous diagonal block (plus the
                                # clamp-added block for q==3) needs masking
                                nc.vector.tensor_tensor(
                                    sp[:, t0c:128 * (q + 1)],
                                    sp[:, t0c:128 * (q + 1)],
                                    mask_t[:, q, t0c:128 * (q + 1)], ADD)
                            strip = spool.tile([P, 512], BF16, tag="strip")
                            nc.scalar.activation(strip[:, t0c:], sp[:, t0c:],
                                                 EXP, bias=0.0, scale=SCALE)
                            nc.tensor.matmul(lsum[:, t0:], ones_sq[:],
                                             strip[:, t0:],
                                             start=(si == 0), stop=(si == ns - 1))
                            nc.tensor.matmul(ztp[:, t0:], vh[:, si, :],
                                             strip[:, t0:],
                                             start=(si == 0), stop=(si == ns - 1))
                        rlb = lrpool.tile([P, 512], F32R, tag="rlb")
                        with nc.allow_low_precision(reason="1/l scale in f32r"):
                            nc.vector.reciprocal(rlb[:], lsum[:])
                        nc.vector.tensor_tensor(
                            zt_tiles[h][:, 512 * tc2:512 * (tc2 + 1)],
                            ztp[:], rlb[:], mybir.AluOpType.mult)

            # ---------------- Phase C: output projection from SBUF ZT
            with (
                tc.tile_pool(name="cps", bufs=CFG["cps"], space="PSUM") as cps,
            ):
                for ti in range(NT):
                    for oc in range(4):
                        ps = cps.tile([P, 512], F32, tag="cps")
                        for h in range(HPC):
                            nc.tensor.matmul(
                                ps[:], zt_tiles[h][:, ti * P:(ti + 1) * P],
                                wo_s[:, h, 512 * oc:512 * (oc + 1)],
                                start=(h == 0), stop=(h == HPC - 1))
                        ev = cev.tile([P, 512], F32, tag="cev")
                        if oc % 2 == 0:
                            nc.vector.tensor_copy(ev[:], ps[:])
                        else:
                            nc.scalar.copy(ev[:], ps[:])
                        nc.sync.dma_start(
                            out[ti * P:(ti + 1) * P, 512 * oc:512 * (oc + 1)],
                            ev[:])


def build(repeat=1, loop_phase=None, phases="ABC", staggered=False):
    nc = bacc.Bacc("TRN2", target_bir_lowering=False, debug=False)
    xT = nc.dram_tensor("xT", [D, T], F32R, kind="ExternalInput").ap()
    wqT = nc.dram_tensor("wqT", [D, 512], F32R, kind="ExternalInput").ap()
    wkT = nc.dram_tensor("wkT", [D, 512], F32R, kind="ExternalInput").ap()
    wvT = nc.dram_tensor("wvT", [D, 512], F32R, kind="ExternalInput").ap()
    woT = nc.dram_tensor("woT", [512, D], F32R, kind="ExternalInput").ap()
    masks = nc.dram_tensor("masks", [P, 4, 512], BF16,
                           kind="ExternalInput").ap()
    ident = nc.dram_tensor("ident", [P, P], F32, kind="ExternalInput").ap()
    out = nc.dram_tensor("out", [T, D], F32, kind="ExternalOutput").ap()
    qk_scr = nc.dram_tensor("qk_scr", [HPC, P, 2 * T], F32R).ap()

    def emit_all():
        with (
            tc.tile_pool(name="h0", bufs=1) as h0pool,
            tc.tile_pool(name="const", bufs=1) as cpool,
        ):
            qk0 = h0pool.tile([P, 2 * T], F32R, name="qk0")
            vh_all = [h0pool.tile([P, NT, P], BF16, tag=f"vh{h}",
                                  name=f"vh{h}") for h in range(HPC)]
            mask_t = cpool.tile([P, 4, 512], BF16)
            ones32 = cpool.tile([P, P], F32)
            nc.vector.memset(ones32[:], 1.0)
            ones_sq = cpool.tile([P, P], BF16)
            nc.vector.tensor_copy(ones_sq[:], ones32[:])
            if "A" in phases:
                _phase_a(tc, nc, xT, wqT, wkT, wvT, qk_scr, qk0, vh_all,
                         mask_t, masks)
            if "B" in phases:
                _phase_bc(tc, nc, mask_t, ones_sq, qk_scr, woT, out,
                          qk0, vh_all)

    with tile.TileContext(nc) as tc:
        if repeat == 1 and loop_phase is None:
            emit_all()
        elif loop_phase is None:
            with tc.For_i(0, repeat, 1, staggered_reset=staggered):
                emit_all()
        else:
            raise ValueError("loop_phase no longer supported")
    nc.compile()
    return nc


def make_inputs(x, Wq, Wk, Wv, Wo):
    """Host-side sharding: returns in_maps for cores 0..7 (core = b*4 + hg)."""
    # transposed mask: strip [s_local, q, t_local]; valid iff sl <= tl - 128*q
    masks = np.full((P, 4, 512), NEG, dtype=np.float32)
    for q in range(4):
        for sl in range(P):
            lo = sl + 128 * q
            if lo < 512:
                masks[sl, q, lo:] = 0.0
    ident = np.eye(P, dtype=np.float32)
    xTs = [np.ascontiguousarray(x[b].T).astype(np.float32) for b in range(2)]
    in_maps = []
    for core in range(8):
        b, hg = core // 4, core % 4
        sl = slice(hg * 512, (hg + 1) * 512)
        in_maps.append({
            "xT": xTs[b],
            "wqT": np.ascontiguousarray(Wq[sl, :].T),
            "wkT": np.ascontiguousarray(Wk[sl, :].T),
            "wvT": np.ascontiguousarray(Wv[sl, :].T),
            "woT": np.ascontiguousarray(Wo[:, sl].T),
            "masks": masks.astype(ml_dtypes.bfloat16),
            "ident": ident,
        })
    return in_maps


_nc_cache = {}


def kernel(x, Wq, Wk, Wv, Wo):
    x = np.asarray(x, dtype=np.float32)
    Wq = np.asarray(Wq, dtype=np.float32)
    Wk = np.asarray(Wk, dtype=np.float32)
    Wv = np.asarray(Wv, dtype=np.float32)
    Wo = np.asarray(Wo, dtype=np.float32)
    if "nc" not in _nc_cache:
        _nc_cache["nc"] = build()
    nc = _nc_cache["nc"]
    in_maps = make_inputs(x, Wq, Wk, Wv, Wo)
    res = run_bass_kernel_spmd(nc, in_maps, core_ids=list(range(8)))
    B = x.shape[0]
    out = np.zeros((B, T, D), dtype=np.float32)
    for core in range(8):
        b = core // 4
        out[b] += res.results[core]["out"]
    return out

